# revision 18
# baseline (speedup 1.0000x reference)
import numpy as np
import ml_dtypes

import concourse.bass as bass
from concourse import mybir
from concourse.bass_utils import run_bass_kernel_spmd
from concourse.tile import TileContext

PI = float(np.pi)
KALPHA = float(np.exp(0.001))  # alpha = ln(k*e^a + k) = softplus(a) + 0.001

B, D, T, E, H = 8192, 1024, 512, 10, 1024
NCORES = 8
BC = B // NCORES  # 1024 rows per core
N = 512           # batch sub-tile (free dim)
NB = BC // N      # 2

F32 = mybir.dt.float32
BF16 = mybir.dt.bfloat16
AF = mybir.ActivationFunctionType
OP = mybir.AluOpType


def _emit(nc, tc, condT, phiT, w1m, b1sD, w2sD, b2sD, outT, ldout):
    with tc.tile_pool(name="persist", bufs=1) as P:
        b1s = P.tile([128, 8], F32, name="b1s", tag="b1s")
        b2s = P.tile([128, 120], F32, name="b2s", tag="b2s")
        ones = P.tile([128, 1], F32, name="ones", tag="ones")
        nc.sync.dma_start(out=b1s[:], in_=b1sD[:, :])
        nc.sync.dma_start(out=b2s[:], in_=b2sD[:, :])
        nc.any.memset(ones[:], 1.0)

        hT = [P.tile([128, BC], BF16, name=f"hT{m}", tag=f"hT{m}") for m in range(8)]

        # ---- Phase 1: features + MM1 -> hT (relu, bf16) ----
        with (
            tc.tile_pool(name="wf", bufs=1) as WF,
            tc.tile_pool(name="wft", bufs=4) as WFT,
            tc.tile_pool(name="ps1", bufs=2, space="PSUM") as PS1,
        ):
            featA = [WF.tile([128, BC], BF16, name=f"fA{s}", tag=f"fA{s}") for s in range(4)]
            featB = [WF.tile([128, BC], BF16, name=f"fB{s}", tag=f"fB{s}") for s in range(4)]
            for sc in range(4):
                ct = WFT.tile([128, BC], F32, name="ct", tag="ct")
                nc.sync.dma_start(out=ct[:], in_=condT[sc * 128:(sc + 1) * 128, :])
                q = WFT.tile([128, BC], F32, name="q", tag="q")
                # q = sin(0.5c - pi/2) = -cos(c/2);  q^2 = (1+cos c)/2
                nc.scalar.activation(q[:], ct[:], AF.Sin, bias=-PI / 2, scale=0.5)
                nc.scalar.activation(featA[sc][:], q[:], AF.Square)
                # sin(c) = -sin(c - pi); sign folded into W1mod rows 512..1023
                nc.scalar.activation(featB[sc][:], ct[:], AF.Sin, bias=-PI)

            w1k = [WF.tile([128, H], BF16, name=f"w1_{k}", tag=f"w1_{k}") for k in range(8)]
            for kc in range(8):
                nc.sync.dma_start(out=w1k[kc][:], in_=w1m[kc * 128:(kc + 1) * 128, :])

            for m in range(8):
                for nb in range(NB):
                    ps = PS1.tile([128, N], F32, name="mm1ps", tag="mm1ps")
                    for kc in range(8):
                        feat = featA[kc] if kc < 4 else featB[kc - 4]
                        nc.tensor.matmul(
                            ps[:],
                            w1k[kc][:, m * 128:(m + 1) * 128],
                            feat[:, nb * N:(nb + 1) * N],
                            start=(kc == 0),
                            stop=(kc == 7),
                        )
                    nc.scalar.activation(
                        hT[m][:, nb * N:(nb + 1) * N], ps[:], AF.Relu,
                        bias=b1s[:, m:m + 1], scale=1.0,
                    )

        # ---- Phases 2+3 ----
        with (
            tc.tile_pool(name="w2st", bufs=3) as W2P,
            tc.tile_pool(name="tmp", bufs=2) as TMP,
            tc.tile_pool(name="psa", bufs=2, space="PSUM") as PSA,
            tc.tile_pool(name="psb", bufs=2, space="PSUM") as PSB,
            tc.tile_pool(name="psr", bufs=2, space="PSUM") as PSR,
            tc.tile_pool(name="psld", bufs=1, space="PSUM") as PSL,
        ):
            # Phase 2: x = tan(0.5(phi-pi)) = -cos(phi/2)/sin(phi/2), x2p1 = 1+x^2 = 1/sin^2
            xs = [P.tile([128, BC], F32, name=f"x{t}", tag=f"x{t}") for t in range(4)]
            x2 = [P.tile([128, BC], F32, name=f"x2{t}", tag=f"x2{t}") for t in range(4)]
            for tcc in range(4):
                pt = TMP.tile([128, BC], F32, name="pt", tag="pt")
                nc.sync.dma_start(out=pt[:], in_=phiT[tcc * 128:(tcc + 1) * 128, :])
                sph = TMP.tile([128, BC], F32, name="sph", tag="sph")
                nc.scalar.activation(sph[:], pt[:], AF.Sin, bias=0.0, scale=0.5)
                cph = TMP.tile([128, BC], F32, name="cph", tag="cph")
                nc.scalar.activation(cph[:], pt[:], AF.Sin, bias=PI / 2, scale=-0.5)
                rs = TMP.tile([128, BC], F32, name="rs", tag="rs")
                nc.vector.reciprocal(out=rs[:], in_=sph[:])
                nc.vector.scalar_tensor_tensor(
                    out=xs[tcc][:], in0=cph[:], scalar=-1.0, in1=rs[:],
                    op0=OP.mult, op1=OP.mult,
                )
                nc.vector.tensor_tensor(x2[tcc][:], rs[:], rs[:], OP.mult)

            accW = [P.tile([128, N], F32, name=f"accW{n}", tag=f"accW{n}") for n in range(NB)]
            accA = [P.tile([128, N], F32, name=f"accA{n}", tag=f"accA{n}") for n in range(NB)]
            accD = [P.tile([128, N], F32, name=f"accD{n}", tag=f"accD{n}") for n in range(NB)]
            ldt = [[P.tile([128, N], F32, name=f"ldt{t}_{n}", tag=f"ldt{t}_{n}")
                    for n in range(NB)] for t in range(4)]
            ldoutS = P.tile([1, BC], F32, name="ldoutS", tag="ldoutS")

            def tile_f32(tag):
                return TMP.tile([128, N], F32, name=tag, tag=tag)

            for tcc in range(4):
                for nb in range(NB):
                    nc.gpsimd.memset(accW[nb][:], 0.0)
                    nc.gpsimd.memset(accA[nb][:], 0.0)
                    nc.gpsimd.memset(accD[nb][:], 0.0)
                for e in range(E):
                    wj = []
                    for j in range(3):
                        w2t = W2P.tile([128, H], BF16, name=f"w2j{j}", tag=f"w2j{j}")
                        r0 = ((e * 3 + j) * 4 + tcc) * 128
                        nc.sync.dma_start(out=w2t[:], in_=w2sD[r0:r0 + 128, :])
                        wj.append(w2t)
                    for nb in range(NB):
                        psA = PSA.tile([128, N], F32, name="psA", tag="psA")
                        psB = PSB.tile([128, N], F32, name="psB", tag="psB")
                        psR = PSR.tile([128, N], F32, name="psR", tag="psR")
                        for j, ps in ((0, psA), (1, psB), (2, psR)):
                            for kc in range(8):
                                nc.tensor.matmul(
                                    ps[:],
                                    wj[j][:, kc * 128:(kc + 1) * 128],
                                    hT[kc][:, nb * N:(nb + 1) * N],
                                    start=(kc == 0),
                                    stop=(kc == 7),
                                )
                        colA = (e * 3 + 0) * 4 + tcc
                        colB = (e * 3 + 1) * 4 + tcc
                        colR = (e * 3 + 2) * 4 + tcc
                        xv = xs[tcc][:, nb * N:(nb + 1) * N]

                        E1 = tile_f32("E1")
                        nc.scalar.activation(E1[:], psA[:], AF.Exp,
                                             bias=b2s[:, colA:colA + 1], scale=1.0)
                        al = tile_f32("al")  # alpha = ln(k*E1 + k) = softplus(araw)+1e-3
                        nc.scalar.activation(al[:], E1[:], AF.Ln,
                                             bias=KALPHA, scale=KALPHA)
                        wE = tile_f32("wE")  # unnormalized softmax weight
                        nc.scalar.activation(wE[:], psR[:], AF.Exp,
                                             bias=b2s[:, colR:colR + 1], scale=1.0)
                        t0 = tile_f32("t0")
                        nc.vector.tensor_tensor(t0[:], al[:], xv, OP.mult)
                        u = tile_f32("u")  # u = alpha*x + (psB + b2)
                        nc.vector.scalar_tensor_tensor(
                            out=u[:], in0=psB[:], scalar=b2s[:, colB:colB + 1],
                            in1=t0[:], op0=OP.add, op1=OP.add)
                        ud = tile_f32("ud")
                        nc.vector.tensor_scalar(ud[:], u[:], 1.0, -1.0, OP.min, OP.max)
                        ru = tile_f32("ru")
                        nc.vector.reciprocal(out=ru[:], in_=u[:])
                        ur = tile_f32("ur")
                        nc.vector.tensor_scalar(ur[:], ru[:], 1.0, -1.0, OP.min, OP.max)
                        h2 = tile_f32("h2")
                        nc.vector.tensor_scalar(h2[:], u[:], 0.0, PI / 2, OP.is_ge, OP.mult)
                        ad = tile_f32("ad")
                        nc.scalar.activation(ad[:], ud[:], AF.Arctan)
                        ar = tile_f32("ar")
                        nc.scalar.activation(ar[:], ur[:], AF.Arctan)
                        # atan(u) = atan(clamp(u)) - atan(clamp(1/u)) + (u>=0)*pi/2 - pi/4
                        # a2 := atan(u) + pi/4 (shift folded into final affine)
                        t1 = tile_f32("t1")
                        nc.vector.tensor_tensor(t1[:], ad[:], ar[:], OP.subtract)
                        a2 = tile_f32("a2")
                        nc.vector.tensor_tensor(a2[:], t1[:], h2[:], OP.add)
                        ca = tile_f32("ca")  # cos(atan u) = sin(pi/2 - (a2-pi/4))
                        nc.scalar.activation(ca[:], a2[:], AF.Sin,
                                             bias=3 * PI / 4, scale=-1.0)
                        c2 = tile_f32("c2")
                        nc.scalar.activation(c2[:], ca[:], AF.Square)
                        ta = tile_f32("ta")
                        nc.vector.tensor_tensor(ta[:], wE[:], a2[:], OP.mult)
                        pd = tile_f32("pd")
                        nc.vector.tensor_tensor(pd[:], wE[:], al[:], OP.mult)
                        p3 = tile_f32("p3")
                        nc.vector.tensor_tensor(p3[:], pd[:], c2[:], OP.mult)
                        nc.gpsimd.tensor_tensor(accW[nb][:], accW[nb][:], wE[:], OP.add)
                        nc.gpsimd.tensor_tensor(accA[nb][:], accA[nb][:], ta[:], OP.add)
                        nc.gpsimd.tensor_tensor(accD[nb][:], accD[nb][:], p3[:], OP.add)

                for nb in range(NB):
                    rW = tile_f32("E1")
                    nc.vector.reciprocal(out=rW[:], in_=accW[nb][:])
                    o1 = tile_f32("al")
                    nc.vector.tensor_tensor(o1[:], accA[nb][:], rW[:], OP.mult)
                    oo = tile_f32("wE")  # out = 2*(accA/accW + pi/4) + pi/2 shift
                    nc.scalar.activation(oo[:], o1[:], AF.Copy, bias=PI / 2, scale=2.0)
                    nc.sync.dma_start(
                        out=outT[tcc * 128:(tcc + 1) * 128, nb * N:(nb + 1) * N],
                        in_=oo[:])
                    dm = tile_f32("t0")
                    nc.vector.tensor_tensor(
                        dm[:], x2[tcc][:, nb * N:(nb + 1) * N], accD[nb][:], OP.mult)
                    dn = tile_f32("u")
                    nc.vector.tensor_tensor(dn[:], dm[:], rW[:], OP.mult)
                    nc.scalar.activation(ldt[tcc][nb][:], dn[:], AF.Ln)

            for nb in range(NB):
                ldps = PSL.tile([1, N], F32, name=f"ldps{nb}", tag=f"ldps{nb}")
                for tcl in range(4):
                    nc.tensor.matmul(ldps[:], ones[:], ldt[tcl][nb][:],
                                     start=(tcl == 0), stop=(tcl == 3))
                nc.scalar.activation(ldoutS[:, nb * N:(nb + 1) * N], ldps[:], AF.Copy)
            nc.sync.dma_start(out=ldout[0:1, :], in_=ldoutS[:])


_PROG = None


def _get_prog():
    global _PROG
    if _PROG is None:
        nc = bass.Bass(target_bir_lowering=False)
        for val in (-PI / 2, -PI, PI / 2, KALPHA, 3 * PI / 4):
            t = nc.alloc_sbuf_tensor(f"const-f32-{val}", [128, 1], F32)
            nc.gpsimd.memset(t.ap(), val)
            nc.const_aps.aps[(F32, val)] = t.ap()
        nc.all_engine_barrier()
        condT = nc.dram_tensor("condT", [T, BC], F32, kind="ExternalInput")
        phiT = nc.dram_tensor("phiT", [T, BC], F32, kind="ExternalInput")
        w1m = nc.dram_tensor("w1m", [2 * T, H], BF16, kind="ExternalInput")
        b1sD = nc.dram_tensor("b1s", [128, 8], F32, kind="ExternalInput")
        w2sD = nc.dram_tensor("w2s", [E * 3 * 4 * 128, H], BF16, kind="ExternalInput")
        b2sD = nc.dram_tensor("b2s", [128, 120], F32, kind="ExternalInput")
        outT = nc.dram_tensor("outT", [T, BC], F32, kind="ExternalOutput")
        ldout = nc.dram_tensor("ldout", [1, BC], F32, kind="ExternalOutput")
        with TileContext(nc) as tc:
            _emit(nc, tc, condT, phiT, w1m, b1sD, w2sD, b2sD, outT, ldout)
        # split multi-wait sync (HW allows 1 wait/instruction, 2 on event sem)
        import bass_rust
        bass_rust.generate_event_semaphores(nc)
        # walrus rejects EVENT_SEMAPHORE_RANGE_CLEAR (ISA wrong length); the
        # end-of-tile-context sem clear is redundant (nrt resets sems per run)
        from concourse import mybir as _mb
        for _blk in nc.m.functions[0].blocks:
            _blk.instructions = [
                i for i in _blk.instructions
                if not (isinstance(i, _mb.InstISA) and i.isa_opcode == 176
                        and (i.sync_info is None or
                             (not i.sync_info.on_wait and
                              not i.sync_info.on_update)))]
        _PROG = nc
    return _PROG


def _host_prep(W1, b1, W2, b2):
    bf = ml_dtypes.bfloat16
    W1c, W1s = W1[:T], W1[T:]
    w1mod = np.ascontiguousarray(
        np.concatenate([2.0 * W1c, -W1s], axis=0)).astype(bf)       # [1024,1024]
    b1mod = (b1 - W1c.sum(axis=0)).astype(np.float32)
    b1s = np.ascontiguousarray(b1mod.reshape(8, 128).T)             # [128,8]
    # w2s rows ((e*3+j)*4+tc)*128+k, cols kc*128+m ; W2[K, t*30+e*3+j], K=kc*128+k, t=tc*128+m
    w2s = np.ascontiguousarray(
        W2.reshape(8, 128, 4, 128, E, 3).transpose(4, 5, 2, 1, 0, 3)
        .reshape(E * 3 * 4 * 128, H)).astype(bf)
    b2s = np.ascontiguousarray(
        b2.reshape(4, 128, E, 3).transpose(1, 2, 3, 0).reshape(128, 120)
    ).astype(np.float32)
    return w1mod, b1s, w2s, b2s


def _build_in_maps(z, W1, b1, W2, b2):
    w1mod, b1s, w2s, b2s = _host_prep(
        np.asarray(W1, np.float32), np.asarray(b1, np.float32),
        np.asarray(W2, np.float32), np.asarray(b2, np.float32))
    in_maps = []
    for c in range(NCORES):
        zc = z[c * BC:(c + 1) * BC]
        in_maps.append({
            "condT": np.ascontiguousarray(zc[:, 0::2].T),
            "phiT": np.ascontiguousarray(zc[:, 1::2].T),
            "w1m": w1mod, "b1s": b1s, "w2s": w2s, "b2s": b2s,
        })
    return in_maps


def _run(z, W1, b1, W2, b2, trace=False):
    z = np.asarray(z, np.float32)
    nc = _get_prog()
    in_maps = _build_in_maps(z, W1, b1, W2, b2)
    res = run_bass_kernel_spmd(nc, in_maps, core_ids=list(range(NCORES)), trace=trace)
    z_new = z.copy()
    log_det = np.empty((B,), np.float32)
    for c in range(NCORES):
        r = res.results[c]
        z_new[c * BC:(c + 1) * BC, 1::2] = np.asarray(r["outT"], np.float32).T
        log_det[c * BC:(c + 1) * BC] = np.asarray(r["ldout"], np.float32).reshape(BC)
    return (z_new, log_det), res


def kernel(z, W1, b1, W2, b2):
    out, _ = _run(z, W1, b1, W2, b2, trace=False)
    return out
